# revision 1
# baseline (speedup 1.0000x reference)
"""MetaLSTMCell Trainium2 kernel: 8 cores on a (batch x 2, hidden x 4) grid.

Core i handles batch rows bi*1024:(bi+1)*1024 (bi = i//4) and hidden columns
hi*256:(hi+1)*256 (hi = i%4) for all 4 gates.

Algebraic fold: the hypernetwork projections (zh/zx/zb) are folded into
effective matrices M_* = d*_w[g,hs,:] @ z*_w_g (computed on device), so the
per-core GEMMs are
    D_* = src_meta @ M_*^T (+bias, folded in as an extra K-chunk)
    W_H = h @ w_h_slice^T, W_X = x @ w_x_slice^T,   y = D_H*W_H + D_X*W_X + D_B
in 16 units of [128 batch x (4 gates * 128 h)] per core (8 batch tiles x
2 h-subtiles), batch-tile-outer so each batch tile's LayerNorm moments
complete early.

LayerNorm is over the full hidden dim: per batch tile, one tiny [128, 8]
AllReduce across the 4 same-batch cores (~7-10us measured) merges the
(sum, sumsq) partials; the normalize/gate phase for tile bt is scheduled two
batch tiles later so the AllReduce latency is hidden and never blocks the
DMA queue. A dummy warm-up collective at kernel start absorbs the CC entry
barrier.

Gate blocks are host-permuted to [i, f, o, g] so sigmoid runs as one
[128,384] activation and tanh as one [128,128].
"""

import sys

sys.path.insert(0, "/opt/trn_rl_repo")

from contextlib import ExitStack

import numpy as np
import concourse.bass as bass
import concourse.mybir as mybir
import concourse.tile as tile
from concourse.bass_utils import run_bass_kernel_spmd

B, IN, H, Z, G = 2048, 1024, 1024, 256, 4
NCORES = 8
BI_W, HI_W = 2, 4          # core grid: batch ways x hidden ways
BSH = B // BI_W            # 1024 batch rows per core
HSH = H // HI_W            # 256 hidden cols per core
HS = 128                   # h-subtile width
NHU = HSH // HS            # 2 h-subtiles per core
N = G * HS                 # 512: unit column width (4 gates x 128)
BT = 128                   # batch tile
NBT = BSH // BT            # 8 batch tiles per core
PERM = (0, 1, 3, 2)        # gate order [i, f, o, g]
BLAG = 2                   # phase_b trails phase_a by this many batch tiles

dt = mybir.dt
AF = mybir.ActivationFunctionType
ALU = mybir.AluOpType
F32, BF16 = dt.float32, dt.bfloat16


def fixup_multi_waits(nc):
    """This toolchain's walrus accepts at most ONE sync wait per instruction;
    Tile emits several. Hoist extras onto same-engine NOPs placed before."""
    for f in nc.m.functions:
        for blk in f.blocks:
            out = []
            changed = False
            for inst in blk.instructions:
                si = getattr(inst, "sync_info", None)
                waits = list(si.on_wait) if si is not None and si.on_wait else []
                if len(waits) > 1:
                    changed = True
                    for k, w in enumerate(waits[:-1]):
                        nop = mybir.InstNoOp(
                            name=f"{inst.name}-waitsplit{k}", ins=[], outs=[]
                        )
                        nop.engine = inst.engine
                        nop.sync_info = mybir.SyncInfo(on_wait=[w], on_update=[])
                        out.append(nop)
                    si.on_wait = [waits[-1]]
                out.append(inst)
            if changed:
                blk.instructions = out


def build():
    nc = bass.Bass(trn_type="TRN2", num_devices=NCORES)
    P = 128

    def din(name, shape):
        return nc.dram_tensor(name, shape, F32, kind="ExternalInput")

    xT = din("xT", [IN, BSH])
    hT = din("hT", [IN, BSH])
    mT = din("mT", [Z, BSH])
    c_s = din("c_s", [BSH, HSH])
    whT = din("whT", [NHU, IN, N])
    wxT = din("wxT", [NHU, IN, N])
    zhw = din("zhw", [G * Z, Z])
    zxw = din("zxw", [G * Z, Z])
    zbw = din("zbw", [G * Z, Z])
    dhwT = din("dhwT", [NHU, G * Z, HS])
    dxwT = din("dxwT", [NHU, G * Z, HS])
    dbwT = din("dbwT", [NHU, G * Z, HS])
    bdh = din("bdh", [NHU, N])
    bdx = din("bdx", [NHU, N])
    dbb = din("dbb", [NHU, N])
    lnw = din("lnw", [NHU, N])
    lnb = din("lnb", [NHU, N])
    hn = nc.dram_tensor("hn", [BSH, HSH], F32, kind="ExternalOutput")
    cn = nc.dram_tensor("cn", [BSH, HSH], F32, kind="ExternalOutput")

    quad_groups = [[0, 1, 2, 3], [4, 5, 6, 7]]

    with tile.TileContext(nc) as tc:
        with tc.tile_pool(name="wres", bufs=1) as wres, \
             tc.tile_pool(name="dram", bufs=1, space="DRAM") as dram, \
             tc.tile_pool(name="stream", bufs=3) as sp, \
             tc.tile_pool(name="stage", bufs=2) as sg, \
             tc.tile_pool(name="ypool", bufs=(BLAG + 2) * NHU) as yp, \
             tc.tile_pool(name="cpool", bufs=BLAG + 2) as cp, \
             tc.tile_pool(name="phb", bufs=3) as pb, \
             tc.tile_pool(name="psd", bufs=3, space="PSUM") as psd, \
             tc.tile_pool(name="psw", bufs=5, space="PSUM") as psw:

            # ---- persistent small tiles
            rep_lnw = wres.tile([P, NHU, N], BF16)
            rep_lnb = wres.tile([P, NHU, N], BF16)
            eps_t = wres.tile([P, 1], F32)
            nc.vector.memset(eps_t[:], 1e-5)
            e0 = wres.tile([P, P], BF16)
            nc.vector.memset(e0[:], 0.0)
            nc.vector.memset(e0[:1, :], 1.0)
            bias3h = wres.tile([P, NHU, N], BF16)
            bias3x = wres.tile([P, NHU, N], BF16)
            bias3b = wres.tile([P, NHU, N], BF16)
            for t_ in (bias3h, bias3x, bias3b):
                nc.vector.memset(t_[:], 0.0)
            whb_r = wres.tile([P, NHU, IN // P, N], BF16)
            wxb_r = wres.tile([P, NHU, IN // P, N], BF16)
            Mh_r = wres.tile([P, NHU, 2, N], BF16)
            Mx_r = wres.tile([P, NHU, 2, N], BF16)
            Mb_r = wres.tile([P, NHU, 2, N], BF16)

            mom_in = dram.tile([BSH, 8], F32)
            mom_out = dram.tile([BSH, 8], F32)
            warm_in = dram.tile([1, 8], F32)
            warm_out = dram.tile([1, 8], F32)

            # warm-up collective: absorbs the CC entry barrier while the
            # weight DMAs stream in
            nc.sync.dma_start(warm_in[:], mom_in[0:1, :])
            nc.gpsimd.collective_compute(
                "AllReduce", ALU.add, replica_groups=quad_groups,
                ins=[warm_in[:]], outs=[warm_out[:]])

            with ExitStack() as pre_ctx:
                pre = pre_ctx.enter_context(tc.tile_pool(name="pre", bufs=1))
                # z weights: load + cast once (shared by both hu)
                zres = {}
                for nm, zw_d in (("h", zhw), ("x", zxw), ("b", zbw)):
                    zbf = pre.tile([P, 2 * G, Z], BF16, name=f"zbf_{nm}",
                                   tag=f"zbf_{nm}")
                    for c2 in range(2):
                        zst = pre.tile([P, G, Z], F32, tag="zstage")
                        nc.sync.dma_start(
                            zst[:],
                            zw_d.ap()[c2 * G * P:(c2 + 1) * G * P, :]
                            .rearrange("(c p) z -> p c z", p=P))
                        nc.scalar.copy(zbf[:, c2 * G:(c2 + 1) * G, :], zst[:])
                    zres[nm] = zbf

                for hu in range(NHU):
                    # main-GEMM weights: stream + cast per K-chunk
                    for (w_d, w_r, tg) in ((whT, whb_r, "wst"),
                                           (wxT, wxb_r, "wst")):
                        for kc in range(IN // P):
                            wst = sg.tile([P, N], F32, tag=tg)
                            nc.sync.dma_start(
                                wst[:],
                                w_d.ap()[hu]
                                .rearrange("(k p) n -> p k n", p=P)[:, kc])
                            nc.scalar.copy(w_r[:, hu, kc], wst[:])

                    for (dw_d, MT, zbf) in ((dhwT, Mh_r, zres["h"]),
                                            (dxwT, Mx_r, zres["x"]),
                                            (dbwT, Mb_r, zres["b"])):
                        dst_ = pre.tile([P, 2 * G, HS], F32, tag="dstage")
                        nc.sync.dma_start(
                            dst_[:],
                            dw_d.ap()[hu].rearrange("(c p) n -> p c n", p=P))
                        dbf = pre.tile([P, 2 * G, HS], BF16, tag="dbf")
                        nc.scalar.copy(dbf[:], dst_[:])
                        for g in range(G):
                            for zmc in range(2):
                                ps = psd.tile([P, HS], F32, tag="psd")
                                for zc in range(2):
                                    nc.tensor.matmul(
                                        ps[:],
                                        zbf[:, g * 2 + zc,
                                            zmc * P:(zmc + 1) * P],
                                        dbf[:, g * 2 + zc],
                                        start=(zc == 0), stop=(zc == 1),
                                    )
                                nc.vector.tensor_copy(
                                    MT[:, hu, zmc, g * HS:(g + 1) * HS], ps[:])

                    for (row_d, b3) in ((bdh, bias3h), (bdx, bias3x),
                                        (dbb, bias3b)):
                        rowt = pre.tile([1, N], F32, tag="rowt")
                        nc.sync.dma_start(rowt[:], row_d.ap()[hu:hu + 1, :])
                        nc.vector.tensor_copy(b3[:1, hu], rowt[:])
                    ones = pre.tile([1, P], F32, tag="ones")
                    nc.vector.memset(ones[:], 1.0)
                    for (row_d, rep) in ((lnw, rep_lnw), (lnb, rep_lnb)):
                        rowt = pre.tile([1, N], F32, tag="rowt")
                        nc.sync.dma_start(rowt[:], row_d.ap()[hu:hu + 1, :])
                        bp = psd.tile([P, N], F32, tag="psd")
                        nc.tensor.matmul(bp[:], ones[:], rowt[:], start=True,
                                         stop=True)
                        nc.vector.tensor_copy(rep[:, hu], bp[:])

            ytiles = {}
            ctiles = {}

            def phase_a(bt):
                bs = slice(bt * BT, (bt + 1) * BT)
                st = sg.tile([P, IN // P, BT], F32, tag="st")
                nc.sync.dma_start(
                    st[:], xT.ap().rearrange("(k p) b -> p k b", p=P)[:, :, bs])
                xb = sp.tile([P, IN // P, BT], BF16, tag="xb")
                nc.vector.tensor_copy(xb[:], st[:])
                st2 = sg.tile([P, IN // P, BT], F32, tag="st")
                nc.sync.dma_start(
                    st2[:], hT.ap().rearrange("(k p) b -> p k b", p=P)[:, :, bs])
                hb = sp.tile([P, IN // P, BT], BF16, tag="hb")
                nc.vector.tensor_copy(hb[:], st2[:])
                st3 = sg.tile([P, Z // P, BT], F32, tag="st3")
                nc.sync.dma_start(
                    st3[:], mT.ap().rearrange("(k p) b -> p k b", p=P)[:, :, bs])
                mb = sp.tile([P, Z // P, BT], BF16, tag="mb")
                nc.gpsimd.tensor_copy(mb[:], st3[:])
                c_t = cp.tile([P, HSH], F32, tag="ct")
                nc.sync.dma_start(c_t[:], c_s[bs, :])
                ctiles[bt] = c_t

                mom = sp.tile([P, 8], F32, tag="mom")
                for hu in range(NHU):
                    WH = psw.tile([P, N], F32, tag="psw")
                    for kc in range(IN // P):
                        nc.tensor.matmul(WH[:], hb[:, kc], whb_r[:, hu, kc],
                                         start=(kc == 0),
                                         stop=(kc == IN // P - 1))
                    WX = psw.tile([P, N], F32, tag="psw")
                    for kc in range(IN // P):
                        nc.tensor.matmul(WX[:], xb[:, kc], wxb_r[:, hu, kc],
                                         start=(kc == 0),
                                         stop=(kc == IN // P - 1))
                    DH = psd.tile([P, N], F32, tag="psd")
                    DX = psd.tile([P, N], F32, tag="psd")
                    DB = psd.tile([P, N], F32, tag="psd")
                    for (D, MT, b3) in ((DH, Mh_r, bias3h), (DX, Mx_r, bias3x),
                                        (DB, Mb_r, bias3b)):
                        for kc in range(Z // P):
                            nc.tensor.matmul(D[:], mb[:, kc], MT[:, hu, kc],
                                             start=(kc == 0), stop=False)
                        nc.tensor.matmul(D[:], e0[:], b3[:, hu], start=False,
                                         stop=True)

                    dh_s = sp.tile([P, N], BF16, tag="dh_s")
                    nc.scalar.copy(dh_s[:], DH[:])
                    dx_s = sp.tile([P, N], BF16, tag="dx_s")
                    nc.scalar.copy(dx_s[:], DX[:])
                    db_s = sp.tile([P, N], BF16, tag="db_s")
                    nc.scalar.copy(db_s[:], DB[:])
                    wh_s = sp.tile([P, N], BF16, tag="wh_s")
                    nc.scalar.copy(wh_s[:], WH[:])
                    wx_s = sp.tile([P, N], BF16, tag="wx_s")
                    nc.scalar.copy(wx_s[:], WX[:])
                    y1 = sp.tile([P, N], BF16, tag="y1")
                    nc.vector.tensor_mul(y1[:], wh_s[:], dh_s[:])
                    y2 = sp.tile([P, N], BF16, tag="y2")
                    nc.vector.tensor_mul(y2[:], wx_s[:], dx_s[:])
                    nc.vector.tensor_add(y1[:], y1[:], y2[:])

                    y = yp.tile([P, N], BF16, tag="y")
                    pm = sp.tile([P, 8], F32, tag="pm")
                    ysq = sp.tile([P, N], BF16, tag="ysq")
                    for g in range(G):
                        gs = slice(g * HS, (g + 1) * HS)
                        nc.vector.scalar_tensor_tensor(
                            y[:, gs], y1[:, gs], 1.0, db_s[:, gs],
                            ALU.mult, ALU.add,
                            accum_out=pm[:, g:g + 1] if hu == 0
                            else mom[:, g:g + 1])
                    nc.gpsimd.tensor_mul(ysq[:], y[:], y[:])
                    for g in range(G):
                        nc.vector.reduce_sum(
                            (pm if hu == 0 else mom)[:, 4 + g:5 + g],
                            ysq[:, g * HS:(g + 1) * HS],
                            axis=mybir.AxisListType.X)
                    ytiles[(bt, hu)] = y
                    if hu == 0:
                        first_pm = pm
                nc.vector.tensor_add(mom[:], mom[:], first_pm[:])
                nc.sync.dma_start(mom_in[bs, :], mom[:])
                nc.gpsimd.collective_compute(
                    "AllReduce", ALU.add, replica_groups=quad_groups,
                    ins=[mom_in[bs, :]], outs=[mom_out[bs, :]])

            def phase_b(bt):
                bs = slice(bt * BT, (bt + 1) * BT)
                gmom = pb.tile([P, 8], F32, tag="gmom")
                nc.sync.dma_start(gmom[:], mom_out[bs, :])
                scl = pb.tile([P, 8], F32, tag="scl")
                nc.vector.tensor_scalar_mul(scl[:], gmom[:], 1.0 / H)
                mu = scl[:, 0:4]
                var = pb.tile([P, 4], F32, tag="var")
                nc.vector.tensor_mul(var[:], mu, mu)
                nc.vector.tensor_sub(var[:], scl[:, 4:8], var[:])
                sq = pb.tile([P, 4], F32, tag="sq")
                nc.scalar.activation(sq[:], var[:], AF.Sqrt, bias=eps_t[:])
                rs = pb.tile([P, 4], F32, tag="rs")
                nc.vector.reciprocal(rs[:], sq[:])
                nmrs = pb.tile([P, 4], F32, tag="nmrs")
                nc.vector.scalar_tensor_tensor(
                    nmrs[:], mu, -1.0, rs[:], ALU.mult, ALU.mult)

                for hu in range(NHU):
                    y = ytiles.pop((bt, hu))
                    vv = pb.tile([P, N], F32, tag="vv")
                    for g in range(G):
                        gs = slice(g * HS, (g + 1) * HS)
                        nc.vector.tensor_scalar(
                            vv[:, gs], y[:, gs], rs[:, g:g + 1],
                            nmrs[:, g:g + 1], op0=ALU.mult, op1=ALU.add)
                    nc.gpsimd.tensor_mul(vv[:], vv[:], rep_lnw[:, hu])
                    nc.gpsimd.tensor_add(vv[:], vv[:], rep_lnb[:, hu])
                    gt = pb.tile([P, N], F32, tag="gt")
                    nc.scalar.activation(gt[:, 0:3 * HS], vv[:, 0:3 * HS],
                                         AF.Sigmoid)
                    nc.scalar.activation(gt[:, 3 * HS:N], vv[:, 3 * HS:N],
                                         AF.Tanh)

                    cs_ = ctiles[bt][:, hu * HS:(hu + 1) * HS]
                    sfc = pb.tile([P, HS], F32, tag="sfc")
                    nc.vector.tensor_mul(sfc[:], gt[:, HS:2 * HS], cs_)
                    sit = pb.tile([P, HS], F32, tag="sit")
                    nc.vector.tensor_mul(sit[:], gt[:, 0:HS], gt[:, 3 * HS:N])
                    cn_t = pb.tile([P, HS], F32, tag="cn_t")
                    nc.vector.tensor_add(cn_t[:], sfc[:], sit[:])
                    tc_t = pb.tile([P, HS], F32, tag="tc_t")
                    nc.scalar.activation(tc_t[:], cn_t[:], AF.Tanh)
                    hn_t = pb.tile([P, HS], F32, tag="hn_t")
                    nc.gpsimd.tensor_mul(hn_t[:], gt[:, 2 * HS:3 * HS],
                                         tc_t[:])
                    hs_cols = slice(hu * HS, (hu + 1) * HS)
                    nc.sync.dma_start(cn[bs, hs_cols], cn_t[:])
                    nc.sync.dma_start(hn[bs, hs_cols], hn_t[:])
                del ctiles[bt]

            # ---- main schedule: phase_b trails by BLAG batch tiles
            for bt in range(NBT):
                phase_a(bt)
                if bt >= BLAG:
                    phase_b(bt - BLAG)
            for bt in range(NBT - BLAG, NBT):
                phase_b(bt)

    fixup_multi_waits(nc)
    return nc


_nc = None


def _get_nc():
    global _nc
    if _nc is None:
        _nc = build()
    return _nc


def make_in_maps(src_x, h, c, src_meta, zh_w, zh_b, zx_w, zx_b, zb_w,
                 dh_w, dx_w, db_w, db_b, w_h, w_x, ln_w, ln_b):
    f32 = np.float32
    asc = np.ascontiguousarray
    perm = list(PERM)
    w_h = w_h[perm]
    w_x = w_x[perm]
    dh_w = dh_w[perm]
    dx_w = dx_w[perm]
    db_w = db_w[perm]
    db_b = db_b[perm]
    ln_w = ln_w[perm]
    ln_b = ln_b[perm]
    zh_w = zh_w.reshape(G, Z, Z)[perm].reshape(G * Z, Z)
    zx_w = zx_w.reshape(G, Z, Z)[perm].reshape(G * Z, Z)
    zb_w = zb_w.reshape(G, Z, Z)[perm].reshape(G * Z, Z)
    zh_b2 = zh_b.reshape(G, Z)[perm]
    zx_b2 = zx_b.reshape(G, Z)[perm]

    xT = asc(src_x.T.astype(f32, copy=False))
    hT = asc(h.T.astype(f32, copy=False))
    mT = asc(src_meta.T.astype(f32, copy=False))

    in_maps = []
    for ci in range(NCORES):
        bi, hi = ci // HI_W, ci % HI_W
        brows = slice(bi * BSH, (bi + 1) * BSH)
        hcols = slice(hi * HSH, (hi + 1) * HSH)

        def per_hu_w(w):
            # [NHU, IN, N]: out[hu][j, g*HS+hh] = w[g, hi*HSH + hu*HS + hh, j]
            sl = w[:, hcols, :]                       # [G, HSH, IN]
            out = np.empty((NHU, IN, N), f32)
            for hu in range(NHU):
                blk = sl[:, hu * HS:(hu + 1) * HS, :]  # [G, HS, IN]
                out[hu] = blk.transpose(2, 0, 1).reshape(IN, N)
            return out

        def per_hu_d(dw):
            # [NHU, G*Z, HS]
            sl = dw[:, hcols, :]                      # [G, HSH, Z]
            out = np.empty((NHU, G * Z, HS), f32)
            for hu in range(NHU):
                blk = sl[:, hu * HS:(hu + 1) * HS, :]  # [G, HS, Z]
                out[hu] = blk.transpose(0, 2, 1).reshape(G * Z, HS)
            return out

        def per_hu_row(v):
            # v: [G, HSH] -> [NHU, N] with [hu][g*HS+hh]
            return asc(v.reshape(G, NHU, HS).transpose(1, 0, 2)
                       .reshape(NHU, N).astype(f32))

        bdh_c = np.einsum("gz,ghz->gh", zh_b2, dh_w[:, hcols, :]).astype(f32)
        bdx_c = np.einsum("gz,ghz->gh", zx_b2, dx_w[:, hcols, :]).astype(f32)

        in_maps.append({
            "xT": asc(xT[:, brows]), "hT": asc(hT[:, brows]),
            "mT": asc(mT[:, brows]),
            "c_s": asc(c[brows, hcols]),
            "whT": per_hu_w(w_h), "wxT": per_hu_w(w_x),
            "zhw": asc(zh_w), "zxw": asc(zx_w), "zbw": asc(zb_w),
            "dhwT": per_hu_d(dh_w), "dxwT": per_hu_d(dx_w),
            "dbwT": per_hu_d(db_w),
            "bdh": per_hu_row(bdh_c), "bdx": per_hu_row(bdx_c),
            "dbb": per_hu_row(db_b[:, hcols]),
            "lnw": per_hu_row(ln_w[:, hcols]),
            "lnb": per_hu_row(ln_b[:, hcols]),
        })
    return in_maps


def run(inputs, trace=False):
    nc = _get_nc()
    in_maps = make_in_maps(**inputs)
    res = run_bass_kernel_spmd(nc, in_maps, core_ids=list(range(NCORES)),
                               trace=trace)
    h_next = np.empty((B, H), np.float32)
    c_next = np.empty((B, H), np.float32)
    for ci in range(NCORES):
        bi, hi = ci // HI_W, ci % HI_W
        brows = slice(bi * BSH, (bi + 1) * BSH)
        hcols = slice(hi * HSH, (hi + 1) * HSH)
        h_next[brows, hcols] = res.results[ci]["hn"]
        c_next[brows, hcols] = res.results[ci]["cn"]
    return (h_next, c_next), res


def kernel(**inputs):
    (h_next, c_next), _ = run(inputs, trace=False)
    return (h_next, c_next)



# revision 14
# speedup vs baseline: 1.8615x; 1.8615x over previous
"""MetaLSTMCell TRN2 kernel v2: pure batch-parallel across 8 cores, no collectives.

Each core owns 256 batch rows (2 tiles of 128) and computes the full hidden
dim (4 gates x 1024 cols, as 8 chunks of 512), so the per-gate LayerNorm is
entirely core-local -- no AllReduce, no CC entry barrier.

Host-side weight preprocessing: the hypernetwork projections are folded into
M_*[g] = contract(z*_w[g], d*_w[g]) so that d_*(b) = src_meta[b] @ M_*[g] +
bias row, and all large operands are pre-quantized to fp8-e4m3 (TRN variant,
max +-240) with power-of-2 scales (w_h/w_x x32, M_* x64). The main GEMMs
(K=1024) and the three meta GEMMs (K=256) run as fp8 DoubleRow matmuls
(2 fp8 weights per PE cell => K=256 per instruction, ~1.44x bf16 rate).
PSUM accumulates f32; per-gate bias rows are folded in as K=1 bf16 matmuls
against a ones row.

y = D_H*W_H + D_X*W_X + D_B is assembled by scalar_tensor_tensor ops reading
PSUM directly (descale fused into the STT scalar), with LayerNorm moment
partials accumulated by tensor_tensor_reduce (sum) on DVE and a Square
activation (sum of squares) on the scalar engine. Normalize + sigmoid/tanh +
gate combine for batch tile 0 are interleaved into batch tile 1's GEMM
stream; only tile 1's epilogue trails the GEMMs.
"""

import sys

sys.path.insert(0, "/opt/trn_rl_repo")

import ml_dtypes
import numpy as np
import concourse.bass as bass
import concourse.mybir as mybir
from concourse.bass_utils import run_bass_kernel_spmd
import concourse.tile as tile

B, IN, H, Z, G = 2048, 1024, 1024, 256, 4
NCORES = 8
BSH = B // NCORES          # 256 batch rows per core
BT = 128                   # batch tile (PE output partitions)
NBT = BSH // BT            # 2 batch tiles per core
CW = 512                   # column chunk width
NC = G * H // CW           # 8 chunks; chunk c = (gate g=c//2, half=c%2)
KC = IN // 256             # 4 DoubleRow K-chunks for the main GEMMs
RS = NC * CW               # 4096: one bias row
SM, SW = 64.0, 32.0        # fp8 pre-scales for M_* and w_h/w_x
PERM = (0, 1, 3, 2)        # gate order [i, f, o, g]

dt = mybir.dt
AF = mybir.ActivationFunctionType
ALU = mybir.AluOpType
DR = mybir.MatmulPerfMode.DoubleRow
F32, BF16, F8 = dt.float32, dt.bfloat16, dt.float8e4

NP_F8 = ml_dtypes.float8_e4m3
NP_BF = ml_dtypes.bfloat16


def fixup_multi_waits(nc):
    """This toolchain's walrus accepts at most ONE sync wait per instruction;
    Tile emits several. Hoist extras onto same-engine NOPs placed before."""
    for f in nc.m.functions:
        for blk in f.blocks:
            out = []
            changed = False
            for inst in blk.instructions:
                si = getattr(inst, "sync_info", None)
                waits = list(si.on_wait) if si is not None and si.on_wait else []
                if len(waits) > 1:
                    changed = True
                    for k, w in enumerate(waits[:-1]):
                        nop = mybir.InstNoOp(
                            name=f"{inst.name}-waitsplit{k}", ins=[], outs=[]
                        )
                        nop.engine = inst.engine
                        nop.sync_info = mybir.SyncInfo(on_wait=[w], on_update=[])
                        out.append(nop)
                    si.on_wait = [waits[-1]]
                out.append(inst)
            if changed:
                blk.instructions = out


def build(fixup=True):
    nc = bass.Bass(trn_type="TRN2", num_devices=NCORES)
    P = 128

    def din(name, shape, dty):
        return nc.dram_tensor(name, shape, dty, kind="ExternalInput")

    xq = din("xq", [NBT, P, KC, 2, BT], F8)
    hq = din("hq", [NBT, P, KC, 2, BT], F8)
    mq = din("mq", [NBT, P, 2, BT], F8)
    cq = din("cq", [NBT, P, 2, CW], BF16)
    whq = din("whq", [NC, P, KC, 2, CW], F8)
    wxq = din("wxq", [NC, P, KC, 2, CW], F8)
    mhq = din("mhq", [NC, P, 2, CW], F8)
    mxq = din("mxq", [NC, P, 2, CW], F8)
    mbq = din("mbq", [NC, P, 2, CW], F8)
    rowq = din("rowq", [1, 3 * RS], BF16)
    lnwq = din("lnwq", [P, RS], BF16)
    lnbq = din("lnbq", [P, RS], BF16)
    hnq = nc.dram_tensor("hnq", [NBT, P, 2, CW], BF16, kind="ExternalOutput")
    cnq = nc.dram_tensor("cnq", [NBT, P, 2, CW], BF16, kind="ExternalOutput")

    from contextlib import ExitStack

    with tile.TileContext(nc) as tc, ExitStack() as st:
        e = st.enter_context

        class pools:
            wres = e(tc.tile_pool(name="wres", bufs=1))
            psA = e(tc.tile_pool(name="psA", bufs=4, space="PSUM"))
            psD = e(tc.tile_pool(name="psD", bufs=4, space="PSUM"))
            yp = e(tc.tile_pool(name="yp", bufs=12))
            up = e(tc.tile_pool(name="up", bufs=2))
            vp = e(tc.tile_pool(name="vp", bufs=2))
            qp = e(tc.tile_pool(name="qp", bufs=2))
            gp = e(tc.tile_pool(name="gp", bufs=10))
            pb = e(tc.tile_pool(name="pb", bufs=3))
            mp = e(tc.tile_pool(name="mp", bufs=2))

        _emit(nc, pools, locals())

    if fixup:
        fixup_multi_waits(nc)
    return nc


def _emit(nc, pl, d):
    P = 128
    xq, hq, mq, cq = d["xq"], d["hq"], d["mq"], d["cq"]
    whq, wxq, mhq, mxq, mbq = d["whq"], d["wxq"], d["mhq"], d["mxq"], d["mbq"]
    rowq, lnwq, lnbq, hnq, cnq = d["rowq"], d["lnwq"], d["lnbq"], d["hnq"], d["cnq"]
    wres, psA, psD = pl.wres, pl.psA, pl.psD
    yp, up, vp, qp, gp, pb, mp = pl.yp, pl.up, pl.vp, pl.qp, pl.gp, pl.pb, pl.mp

    # ---- persistent small tiles
    ones = wres.tile([1, P], BF16, name="ones")
    nc.vector.memset(ones[:], 1.0)
    # y is carried at a global scale of SM*SW (LayerNorm is scale-invariant,
    # so no descale ops are needed anywhere -- only eps must be rescaled).
    eps_t = wres.tile([P, 1], F32, name="eps")
    nc.vector.memset(eps_t[:], (SM * SW) ** 2 * 1e-5)

    # ---- activation + small DMAs first (tiny, unblock the first GEMMs)
    rows_t = wres.tile([1, 3 * RS], BF16, name="rows")
    nc.sync.dma_start(rows_t[:], rowq.ap()[:])
    xb, hb, mb_, cb = {}, {}, {}, {}
    for bt in range(NBT):
        hb[bt] = wres.tile([P, KC, 2, BT], F8, name=f"hb{bt}")
        nc.sync.dma_start(hb[bt][:], hq.ap()[bt])
        xb[bt] = wres.tile([P, KC, 2, BT], F8, name=f"xb{bt}")
        nc.sync.dma_start(xb[bt][:], xq.ap()[bt])
        mb_[bt] = wres.tile([P, 2, BT], F8, name=f"mb{bt}")
        nc.sync.dma_start(mb_[bt][:], mq.ap()[bt])

    # ---- weight DMAs in consumption order; c/ln/gate tables mid-stream
    whb, wxb, mhb, mxb, mbb = {}, {}, {}, {}, {}
    lnw_r = wres.tile([P, NC, CW], BF16, name="lnw_r")
    lnb_r = wres.tile([P, NC, CW], BF16, name="lnb_r")

    def chunk_dmas(c):
        whb[c] = wres.tile([P, KC, 2, CW], F8, name=f"whb{c}")
        nc.sync.dma_start(whb[c][:], whq.ap()[c])
        wxb[c] = wres.tile([P, KC, 2, CW], F8, name=f"wxb{c}")
        nc.sync.dma_start(wxb[c][:], wxq.ap()[c])
        for (dst, src, nm) in ((mhb, mhq, "mh"), (mxb, mxq, "mx"),
                               (mbb, mbq, "mb")):
            dst[c] = wres.tile([P, 2, CW], F8, name=f"{nm}b{c}")
            nc.sync.dma_start(dst[c][:], src.ap()[c])

    for c in range(3):
        chunk_dmas(c)
    for bt in range(NBT):
        cb[bt] = wres.tile([P, 2, CW], BF16, name=f"cb{bt}")
        nc.sync.dma_start(cb[bt][:], cq.ap()[bt])
    nc.sync.dma_start(lnw_r[:], lnwq.ap().rearrange("p (c w) -> p c w", w=CW))
    nc.sync.dma_start(lnb_r[:], lnbq.ap().rearrange("p (c w) -> p c w", w=CW))
    for c in range(3, NC):
        chunk_dmas(c)

    ytiles, gts, moms, rss, nmrss = {}, {}, {}, {}, {}

    def gemm_chunk(bt, c):
        WH = psA.tile([P, CW], F32, tag="pw")
        for kc in range(KC):
            nc.tensor.matmul(WH[:], hb[bt][:, kc], whb[c][:, kc],
                             start=(kc == 0), stop=(kc == KC - 1),
                             perf_mode=DR)
        WX = psA.tile([P, CW], F32, tag="pw")
        for kc in range(KC):
            nc.tensor.matmul(WX[:], xb[bt][:, kc], wxb[c][:, kc],
                             start=(kc == 0), stop=(kc == KC - 1),
                             perf_mode=DR)
        Dt = []
        for (j, mt) in enumerate((mhb, mxb, mbb)):
            Dj = psD.tile([P, CW], F32, tag="pd")
            nc.tensor.matmul(Dj[:], mb_[bt][:], mt[c][:],
                             start=True, stop=False, perf_mode=DR)
            nc.tensor.matmul(Dj[:], ones[:1, :],
                             rows_t[:1, j * RS + c * CW:j * RS + (c + 1) * CW],
                             start=False, stop=True)
            Dt.append(Dj)
        DH, DX, DB = Dt
        # DVE may read at most ONE input from PSUM per instruction and
        # GpSimd cannot touch PSUM at all: ScalarE stages WH/WX into SBUF,
        # DVE pairs each with its PSUM-resident D factor.
        wh_s = up.tile([P, CW], BF16, tag="whs")
        nc.scalar.copy(wh_s[:], WH[:])
        wx_s = vp.tile([P, CW], BF16, tag="wxs")
        nc.scalar.copy(wx_s[:], WX[:])
        u = up.tile([P, CW], F32, tag="u")
        nc.vector.tensor_mul(u[:], DH[:], wh_s[:])
        v = vp.tile([P, CW], F32, tag="v")
        nc.vector.tensor_mul(v[:], DX[:], wx_s[:])
        nc.vector.scalar_tensor_tensor(u[:], DB[:], 1.0, u[:],
                                       ALU.mult, ALU.add)
        y = yp.tile([P, CW], BF16, tag="y")
        nc.vector.scalar_tensor_tensor(y[:], u[:], 0.0, v[:],
                                       ALU.add, ALU.add,
                                       accum_out=moms[bt][:, c:c + 1])
        ysq = qp.tile([P, CW], BF16, tag="ysq")
        nc.gpsimd.tensor_mul(ysq[:], y[:], y[:])
        nc.vector.reduce_sum(moms[bt][:, 8 + c:9 + c], ysq[:],
                             axis=mybir.AxisListType.X)
        ytiles[(bt, c)] = y

    def mb_start(bt):
        momt = moms[bt]
        S = pb.tile([P, 8], F32, tag="S")
        nc.vector.tensor_add(S[:, 0:4], momt[:, 0:8:2], momt[:, 1:8:2])
        nc.vector.tensor_add(S[:, 4:8], momt[:, 8:16:2], momt[:, 9:16:2])
        scl = pb.tile([P, 8], F32, tag="scl")
        nc.vector.tensor_scalar_mul(scl[:], S[:], 1.0 / H)
        mu = scl[:, 0:4]
        var = pb.tile([P, 4], F32, tag="var")
        nc.vector.tensor_mul(var[:], mu, mu)
        nc.vector.tensor_sub(var[:], scl[:, 4:8], var[:])
        sq = pb.tile([P, 4], F32, tag="sq")
        nc.scalar.activation(sq[:], var[:], AF.Sqrt, bias=eps_t[:])
        rs = pb.tile([P, 4], F32, tag="rs")
        nc.vector.reciprocal(rs[:], sq[:])
        nmrs = pb.tile([P, 4], F32, tag="nmrs")
        nc.vector.scalar_tensor_tensor(nmrs[:], mu, -1.0, rs[:],
                                       ALU.mult, ALU.mult)
        rss[bt], nmrss[bt] = rs, nmrs

    def mb_chunk(bt, c):
        g = c // 2
        rs, nmrs = rss[bt], nmrss[bt]
        y = ytiles.pop((bt, c))
        vv = pb.tile([P, CW], BF16, tag="vv")
        nc.vector.tensor_scalar(vv[:], y[:], rs[:, g:g + 1], nmrs[:, g:g + 1],
                                op0=ALU.mult, op1=ALU.add)
        nc.gpsimd.tensor_mul(vv[:], vv[:], lnw_r[:, c])
        nc.gpsimd.tensor_add(vv[:], vv[:], lnb_r[:, c])
        gt_t = gp.tile([P, CW], BF16, tag="gt")
        nc.scalar.activation(gt_t[:], vv[:], AF.Sigmoid if g < 3 else AF.Tanh)
        gts[(bt, c)] = gt_t

    def mb_half(bt, half):
        i_t = gts.pop((bt, 0 + half))
        f_t = gts.pop((bt, 2 + half))
        o_t = gts.pop((bt, 4 + half))
        q_t = gts.pop((bt, 6 + half))
        sfc = pb.tile([P, CW], BF16, tag="sfc")
        nc.vector.tensor_mul(sfc[:], f_t[:], cb[bt][:, half])
        sit = pb.tile([P, CW], BF16, tag="sit")
        nc.gpsimd.tensor_mul(sit[:], i_t[:], q_t[:])
        cnt = pb.tile([P, CW], BF16, tag="cnt")
        nc.vector.tensor_add(cnt[:], sfc[:], sit[:])
        nc.sync.dma_start(cnq.ap()[bt][:, half], cnt[:])
        tct = pb.tile([P, CW], BF16, tag="tct")
        nc.scalar.activation(tct[:], cnt[:], AF.Tanh)
        hnt = pb.tile([P, CW], BF16, tag="hnt")
        nc.gpsimd.tensor_mul(hnt[:], o_t[:], tct[:])
        nc.sync.dma_start(hnq.ap()[bt][:, half], hnt[:])

    # ---- main schedule
    for bt in range(NBT):
        moms[bt] = mp.tile([P, 16], F32, tag="mom", name=f"mom{bt}")
        if bt == 1:
            mb_start(0)
        for c in range(NC):
            gemm_chunk(bt, c)
            if bt == 1:
                mb_chunk(0, c)
                if c == NC - 2:
                    mb_half(0, 0)
                if c == NC - 1:
                    mb_half(0, 1)
    mb_start(1)
    for c in range(NC):
        mb_chunk(1, c)
    mb_half(1, 0)
    mb_half(1, 1)


_nc = None


def _get_nc():
    global _nc
    if _nc is None:
        _nc = build()
    return _nc


def _pack_k(a):
    """[K, C] -> [128, K//256, 2, C] with k = kc*256 + 2p + i (DoubleRow)."""
    K, C = a.shape
    return np.ascontiguousarray(
        a.reshape(K // 256, 128, 2, C).transpose(1, 0, 2, 3))


def _q8(a):
    return np.clip(a, -240.0, 240.0).astype(NP_F8)


def _row_flat(v):
    """[G, H] -> [RS] in chunk-major order (c = g*2 + half)."""
    return np.ascontiguousarray(v.reshape(G * 2, CW).reshape(-1))


def make_in_maps(src_x, h, c, src_meta, zh_w, zh_b, zx_w, zx_b, zb_w,
                 dh_w, dx_w, db_w, db_b, w_h, w_x, ln_w, ln_b):
    f32 = np.float32
    perm = list(PERM)
    w_h = w_h[perm]
    w_x = w_x[perm]
    dh_w = dh_w[perm]
    dx_w = dx_w[perm]
    db_w = db_w[perm]
    db_b = db_b[perm]
    ln_w = ln_w[perm]
    ln_b = ln_b[perm]
    zh3 = zh_w.reshape(G, Z, Z)[perm]
    zx3 = zx_w.reshape(G, Z, Z)[perm]
    zb3 = zb_w.reshape(G, Z, Z)[perm]
    zh_b2 = zh_b.reshape(G, Z)[perm]
    zx_b2 = zx_b.reshape(G, Z)[perm]

    # hypernetwork fold: D_*(b) = src_meta[b] @ M_*[g] + bias row
    M_h = np.einsum("gzy,ghz->gyh", zh3, dh_w).astype(f32)
    M_x = np.einsum("gzy,ghz->gyh", zx3, dx_w).astype(f32)
    M_b = np.einsum("gzy,ghz->gyh", zb3, db_w).astype(f32)
    bdh = np.einsum("gz,ghz->gh", zh_b2, dh_w).astype(f32)
    bdx = np.einsum("gz,ghz->gh", zx_b2, dx_w).astype(f32)

    # replicated (per-core-identical) weight uploads
    def wpack(w):
        out = np.empty((NC, 128, KC, 2, CW), f32)
        for cidx in range(NC):
            g, half = cidx // 2, cidx % 2
            blk = w[g, half * CW:(half + 1) * CW, :]          # [CW, IN]
            out[cidx] = _pack_k(np.ascontiguousarray(blk.T))  # [IN, CW] packed
        return _q8(out * SW)

    def mpack(M, scale):
        out = np.empty((NC, 128, 2, CW), f32)
        for cidx in range(NC):
            g, half = cidx // 2, cidx % 2
            out[cidx] = _pack_k(M[g][:, half * CW:(half + 1) * CW])[:, 0]
        return _q8(out * scale)

    whq = wpack(w_h)
    wxq = wpack(w_x)
    mhq = mpack(M_h, SM)
    mxq = mpack(M_x, SM)
    mbq = mpack(M_b, SM * SW)
    # D_H/D_X rows ride at scale SM (they multiply W at scale SW -> SM*SW);
    # the additive D_B path must carry the full SM*SW scale itself.
    rowq = np.concatenate([_row_flat(bdh * SM), _row_flat(bdx * SM),
                           _row_flat((db_b * SM * SW).astype(f32))])[None, :] \
        .astype(NP_BF)
    lnwq = np.ascontiguousarray(
        np.broadcast_to(_row_flat(ln_w)[None, :], (128, RS))).astype(NP_BF)
    lnbq = np.ascontiguousarray(
        np.broadcast_to(_row_flat(ln_b)[None, :], (128, RS))).astype(NP_BF)

    xT = np.ascontiguousarray(src_x.T.astype(f32, copy=False))
    hT = np.ascontiguousarray(h.T.astype(f32, copy=False))
    mT = np.ascontiguousarray(src_meta.T.astype(f32, copy=False))

    in_maps = []
    for ci in range(NCORES):
        r0 = ci * BSH

        def actpack(aT):  # [K, B] slice -> [NBT, 128, K//256, 2, BT] fp8
            out = np.empty((NBT, 128, aT.shape[0] // 256, 2, BT), f32)
            for bt in range(NBT):
                out[bt] = _pack_k(aT[:, r0 + bt * BT:r0 + (bt + 1) * BT])
            return _q8(out)

        c_sl = c[r0:r0 + BSH].reshape(NBT, 128, 2, CW)
        in_maps.append({
            "xq": actpack(xT), "hq": actpack(hT),
            "mq": actpack(mT)[:, :, 0],
            "cq": c_sl.astype(NP_BF),
            "whq": whq, "wxq": wxq,
            "mhq": mhq, "mxq": mxq, "mbq": mbq,
            "rowq": rowq, "lnwq": lnwq, "lnbq": lnbq,
        })
    return in_maps


def run(inputs, trace=False):
    nc = _get_nc()
    in_maps = make_in_maps(**inputs)
    res = run_bass_kernel_spmd(nc, in_maps, core_ids=list(range(NCORES)),
                               trace=trace)
    h_next = np.empty((B, H), np.float32)
    c_next = np.empty((B, H), np.float32)
    for ci in range(NCORES):
        rows = slice(ci * BSH, (ci + 1) * BSH)
        h_next[rows] = res.results[ci]["hnq"].reshape(BSH, H).astype(np.float32)
        c_next[rows] = res.results[ci]["cnq"].reshape(BSH, H).astype(np.float32)
    return (h_next, c_next), res


def kernel(**inputs):
    (h_next, c_next), _ = run(inputs, trace=False)
    return (h_next, c_next)


# revision 24
# speedup vs baseline: 2.3157x; 1.2440x over previous
"""MetaLSTMCell TRN2 kernel v3: pure batch-parallel across 8 cores, no collectives.

Each core owns 256 batch rows (2 tiles of 128) and computes the full hidden
dim (4 gates x 1024 cols, as 8 chunks of 512), so the per-gate LayerNorm is
entirely core-local -- no AllReduce, no CC entry barrier.

Host-side weight preprocessing: the hypernetwork projections are folded into
M_*[g] = contract(z*_w[g], d*_w[g]) so that d_*(b) = src_meta[b] @ M_*[g] +
bias row, and all large operands are pre-quantized to fp8-e4m3 (TRN variant,
max +-240) with power-of-2 scales (w_h/w_x x32, M_h/M_x x64, M_b x2048).
y rides at a single global scale SM*SW everywhere; LayerNorm is
scale-invariant so no descales are needed (only eps is rescaled).

GEMMs run as fp8 DoubleRow matmuls (K=256 per instruction). Per-gate bias
rows are folded in as K=1 bf16 matmuls against a ones row. Engine split per
512-col chunk (measured ~0.6-0.7us per [128,512] op on DVE/ScalarE, ~1.2us
on GpSimd, which also cannot read PSUM):
  ScalarE: stage WH/WX PSUM->SBUF (2 copies) + Square(y) with accum (sumsq)
  DVE:     u=DH*wh_s, v=DX*wx_s, t=DB+u  (the three PSUM-reading ops)
  GpSimd:  y = t + v with accum (sum)    (SBUF-only)
The LayerNorm affine (nmrs x lnw + lnb, a rank-2 [128,512] map) is built by
a K=5 matmul against a host-built masked table (RZ), using an on-device
transpose of nmrs -- phase_b is then 2 DVE ops + 1 activation per chunk.
Phase_b of batch tile 0 interleaves into tile 1's GEMM stream.

DMA: weights stream on both HWDGE queues (wh on SyncE, wx+M3 on ScalarE) in
consumption order; activation tables are preloaded with dummy ops at t=0.
"""

import sys

sys.path.insert(0, "/opt/trn_rl_repo")

import ml_dtypes
import numpy as np
import concourse.bass as bass
import concourse.mybir as mybir
from concourse.bass_utils import run_bass_kernel_spmd
import concourse.tile as tile

B, IN, H, Z, G = 2048, 1024, 1024, 256, 4
NCORES = 8
BSH = B // NCORES          # 256 batch rows per core
BT = 128                   # batch tile (PE output partitions)
NBT = BSH // BT            # 2 batch tiles per core
CW = 512                   # column chunk width
NC = G * H // CW           # 8 chunks; chunk c = (gate g=c//2, half=c%2)
KC = IN // 256             # 4 DoubleRow K-chunks for the main GEMMs
RS = NC * CW               # 4096: one bias row
SM, SW = 64.0, 32.0        # fp8 pre-scales for M_h/M_x and w_h/w_x
PERM = (0, 1, 3, 2)        # gate order [i, f, o, g]

dt = mybir.dt
AF = mybir.ActivationFunctionType
ALU = mybir.AluOpType
DR = mybir.MatmulPerfMode.DoubleRow
F32, BF16, F8 = dt.float32, dt.bfloat16, dt.float8e4

NP_F8 = ml_dtypes.float8_e4m3
NP_BF = ml_dtypes.bfloat16


def fixup_multi_waits(nc):
    """This toolchain's walrus accepts at most ONE sync wait per instruction;
    Tile emits several. Hoist extras onto same-engine NOPs placed before."""
    for f in nc.m.functions:
        for blk in f.blocks:
            out = []
            changed = False
            for inst in blk.instructions:
                si = getattr(inst, "sync_info", None)
                waits = list(si.on_wait) if si is not None and si.on_wait else []
                if len(waits) > 1:
                    changed = True
                    for k, w in enumerate(waits[:-1]):
                        nop = mybir.InstNoOp(
                            name=f"{inst.name}-waitsplit{k}", ins=[], outs=[]
                        )
                        nop.engine = inst.engine
                        nop.sync_info = mybir.SyncInfo(on_wait=[w], on_update=[])
                        out.append(nop)
                    si.on_wait = [waits[-1]]
                out.append(inst)
            if changed:
                blk.instructions = out


def build(fixup=True):
    nc = bass.Bass(trn_type="TRN2", num_devices=NCORES)

    def din(name, shape, dty):
        return nc.dram_tensor(name, shape, dty, kind="ExternalInput")

    P = 128
    xq = din("xq", [NBT, P, KC, 2, BT], F8)
    hq = din("hq", [NBT, P, KC, 2, BT], F8)
    mq = din("mq", [NBT, P, 2, BT], F8)
    cq = din("cq", [NBT, P, 2, CW], BF16)
    whq = din("whq", [NC, P, KC, 2, CW], F8)
    wxq = din("wxq", [NC, P, KC, 2, CW], F8)
    m3q = din("m3q", [NC, P, 3, 2, CW], F8)
    rowq = din("rowq", [1, 3 * RS], BF16)
    lnwq = din("lnwq", [P, RS], BF16)
    rzq = din("rzq", [5, RS], BF16)
    idq = din("idq", [P, P], BF16)
    hnq = nc.dram_tensor("hnq", [NBT, P, 2, CW], BF16, kind="ExternalOutput")
    cnq = nc.dram_tensor("cnq", [NBT, P, 2, CW], BF16, kind="ExternalOutput")

    from contextlib import ExitStack

    with tile.TileContext(nc) as tc, ExitStack() as st:
        e = st.enter_context

        class pools:
            wres = e(tc.tile_pool(name="wres", bufs=1))
            psW = e(tc.tile_pool(name="psW", bufs=3, space="PSUM"))
            psD = e(tc.tile_pool(name="psD", bufs=3, space="PSUM"))
            psF = e(tc.tile_pool(name="psF", bufs=2, space="PSUM"))
            yp = e(tc.tile_pool(name="yp", bufs=12))
            up = e(tc.tile_pool(name="up", bufs=3))
            vp = e(tc.tile_pool(name="vp", bufs=3))
            qp = e(tc.tile_pool(name="qp", bufs=2))
            gp = e(tc.tile_pool(name="gp", bufs=10))
            pb = e(tc.tile_pool(name="pb", bufs=3))
            mp = e(tc.tile_pool(name="mp", bufs=2))

        _emit(nc, pools, locals())

    if fixup:
        fixup_multi_waits(nc)
    return nc


def _emit(nc, pl, d):
    P = 128
    xq, hq, mq, cq = d["xq"], d["hq"], d["mq"], d["cq"]
    whq, wxq, m3q = d["whq"], d["wxq"], d["m3q"]
    rowq, lnwq, rzq, idq = d["rowq"], d["lnwq"], d["rzq"], d["idq"]
    hnq, cnq = d["hnq"], d["cnq"]
    wres, psW, psD, psF = pl.wres, pl.psW, pl.psD, pl.psF
    yp, up, vp, qp, gp, pb, mp = pl.yp, pl.up, pl.vp, pl.qp, pl.gp, pl.pb, pl.mp

    # ---- persistent small tiles + activation-table preloads
    ones = wres.tile([1, P], BF16, name="ones")
    nc.vector.memset(ones[:], 1.0)
    eps_t = wres.tile([P, 1], F32, name="eps")
    nc.vector.memset(eps_t[:], (SM * SW) ** 2 * 1e-5)
    dum = wres.tile([P, 1], F32, name="dum")
    for fn in (AF.Square, AF.Sigmoid, AF.Tanh, AF.Sqrt):
        nc.scalar.activation(dum[:], eps_t[:], fn)

    # ---- sync-queue DMAs: activations, tables, then wh chunks in order
    rows_t = wres.tile([1, 3 * RS], BF16, name="rows")
    nc.sync.dma_start(rows_t[:], rowq.ap()[:])
    lnw_r = wres.tile([P, NC, CW], BF16, name="lnw_r")
    xb, hb, mb_, cb = {}, {}, {}, {}
    for bt in range(NBT):
        hb[bt] = wres.tile([P, KC, 2, BT], F8, name=f"hb{bt}")
        nc.sync.dma_start(hb[bt][:], hq.ap()[bt])
        xb[bt] = wres.tile([P, KC, 2, BT], F8, name=f"xb{bt}")
        nc.sync.dma_start(xb[bt][:], xq.ap()[bt])
        mb_[bt] = wres.tile([P, 2, BT], F8, name=f"mb{bt}")
        nc.sync.dma_start(mb_[bt][:], mq.ap()[bt])
    rz = wres.tile([5, NC, CW], BF16, name="rz")
    nc.sync.dma_start(rz[:], rzq.ap().rearrange("p (c w) -> p c w", w=CW))
    idt = wres.tile([P, P], BF16, name="idt")
    nc.sync.dma_start(idt[:], idq.ap()[:])

    whb, wxb, m3b = {}, {}, {}
    for c in range(NC):
        whb[c] = wres.tile([P, KC, 2, CW], F8, name=f"whb{c}")
        nc.sync.dma_start(whb[c][:], whq.ap()[c])
        if c == 2:
            for bt in range(NBT):
                cb[bt] = wres.tile([P, 2, CW], BF16, name=f"cb{bt}")
                nc.sync.dma_start(cb[bt][:], cq.ap()[bt])
        if c == 3:
            nc.sync.dma_start(
                lnw_r[:], lnwq.ap().rearrange("p (c w) -> p c w", w=CW))

    # ---- scalar-queue DMAs: wx + M3 chunks (second HWDGE queue)
    def chunk_dmas_scalar(c):
        wxb[c] = wres.tile([P, KC, 2, CW], F8, name=f"wxb{c}")
        nc.scalar.dma_start(wxb[c][:], wxq.ap()[c])
        m3b[c] = wres.tile([P, 3, 2, CW], F8, name=f"m3b{c}")
        nc.scalar.dma_start(m3b[c][:], m3q.ap()[c])

    for c in range(4):
        chunk_dmas_scalar(c)

    ytiles, gts, moms, rss, lts = {}, {}, {}, {}, {}

    def gemm_chunk(bt, c):
        if bt == 0 and c < 4:
            chunk_dmas_scalar(c + 4)
        WH = psW.tile([P, CW], F32, tag="pw")
        for kc in range(KC):
            nc.tensor.matmul(WH[:], hb[bt][:, kc], whb[c][:, kc],
                             start=(kc == 0), stop=(kc == KC - 1),
                             perf_mode=DR)
        WX = psW.tile([P, CW], F32, tag="pw")
        for kc in range(KC):
            nc.tensor.matmul(WX[:], xb[bt][:, kc], wxb[c][:, kc],
                             start=(kc == 0), stop=(kc == KC - 1),
                             perf_mode=DR)
        Dt = []
        for j in range(3):
            Dj = psD.tile([P, CW], F32, tag="pd")
            nc.tensor.matmul(Dj[:], mb_[bt][:], m3b[c][:, j],
                             start=True, stop=False, perf_mode=DR)
            nc.tensor.matmul(Dj[:], ones[:1, :],
                             rows_t[:1, j * RS + c * CW:j * RS + (c + 1) * CW],
                             start=False, stop=True)
            Dt.append(Dj)
        DH, DX, DB = Dt
        wh_s = up.tile([P, CW], BF16, tag="whs")
        nc.scalar.copy(wh_s[:], WH[:])
        wx_s = vp.tile([P, CW], BF16, tag="wxs")
        nc.scalar.copy(wx_s[:], WX[:])
        u = up.tile([P, CW], BF16, tag="u")
        nc.vector.tensor_mul(u[:], DH[:], wh_s[:])
        v = vp.tile([P, CW], BF16, tag="v")
        nc.vector.tensor_mul(v[:], DX[:], wx_s[:])
        t = up.tile([P, CW], BF16, tag="t")
        nc.vector.scalar_tensor_tensor(t[:], DB[:], 1.0, u[:],
                                       ALU.mult, ALU.add)
        y = yp.tile([P, CW], BF16, tag="y")
        nc.vector.scalar_tensor_tensor(y[:], t[:], 0.0, v[:],
                                       ALU.add, ALU.add,
                                       accum_out=moms[bt][:, c:c + 1])
        ysq = qp.tile([P, CW], BF16, tag="ysq")
        nc.scalar.activation(ysq[:], y[:], AF.Square,
                             accum_out=moms[bt][:, 8 + c:9 + c])
        ytiles[(bt, c)] = y

    def mb_start(bt):
        momt = moms[bt]
        S = pb.tile([P, 8], F32, tag="S")
        nc.vector.tensor_add(S[:, 0:4], momt[:, 0:8:2], momt[:, 1:8:2])
        nc.vector.tensor_add(S[:, 4:8], momt[:, 8:16:2], momt[:, 9:16:2])
        scl = pb.tile([P, 8], F32, tag="scl")
        nc.vector.tensor_scalar_mul(scl[:], S[:], 1.0 / H)
        mu = scl[:, 0:4]
        var = pb.tile([P, 4], F32, tag="var")
        nc.vector.tensor_mul(var[:], mu, mu)
        nc.vector.tensor_sub(var[:], scl[:, 4:8], var[:])
        sq = pb.tile([P, 4], F32, tag="sq")
        nc.scalar.activation(sq[:], var[:], AF.Sqrt, bias=eps_t[:])
        rs = pb.tile([P, 4], F32, tag="rs")
        nc.vector.reciprocal(rs[:], sq[:])
        nmrs = pb.tile([P, 4], BF16, tag="nmrs")
        nc.vector.scalar_tensor_tensor(nmrs[:], mu, -1.0, rs[:],
                                       ALU.mult, ALU.mult)
        # LT = [nmrs.T (4 rows); ones]: the K=5 stationary for the affine mm
        tp = psF.tile([4, P], BF16, tag="aff", name=f"tp{bt}")
        nc.tensor.transpose(tp[:], nmrs[:], idt[:])
        lt = pb.tile([5, P], BF16, tag="lt")
        nc.vector.memset(lt[:], 1.0)
        nc.scalar.copy(lt[0:4, :], tp[:])
        rss[bt], lts[bt] = rs, lt

    def mb_chunk(bt, c):
        g = c // 2
        y = ytiles.pop((bt, c))
        # AFF[r, n] = nmrs[r, g]*lnw[n] + lnb[n]  (RZ masks the other gates)
        aff = psF.tile([P, CW], F32, tag="aff")
        nc.tensor.matmul(aff[:], lts[bt][:], rz[:, c], start=True, stop=True)
        s0 = pb.tile([P, CW], BF16, tag="s0")
        nc.vector.scalar_tensor_tensor(s0[:], y[:], rss[bt][:, g:g + 1],
                                       lnw_r[:, c], ALU.mult, ALU.mult)
        vv = pb.tile([P, CW], BF16, tag="vv")
        nc.vector.scalar_tensor_tensor(vv[:], aff[:], 1.0, s0[:],
                                       ALU.mult, ALU.add)
        gt_t = gp.tile([P, CW], BF16, tag="gt")
        nc.scalar.activation(gt_t[:], vv[:], AF.Sigmoid if g < 3 else AF.Tanh)
        gts[(bt, c)] = gt_t

    def mb_half(bt, half):
        i_t = gts.pop((bt, 0 + half))
        f_t = gts.pop((bt, 2 + half))
        o_t = gts.pop((bt, 4 + half))
        q_t = gts.pop((bt, 6 + half))
        sfc = pb.tile([P, CW], BF16, tag="sfc")
        nc.gpsimd.tensor_mul(sfc[:], f_t[:], cb[bt][:, half])
        sit = pb.tile([P, CW], BF16, tag="sit")
        nc.vector.tensor_mul(sit[:], i_t[:], q_t[:])
        cnt = pb.tile([P, CW], BF16, tag="cnt")
        nc.vector.tensor_add(cnt[:], sfc[:], sit[:])
        nc.sync.dma_start(cnq.ap()[bt][:, half], cnt[:])
        tct = pb.tile([P, CW], BF16, tag="tct")
        nc.scalar.activation(tct[:], cnt[:], AF.Tanh)
        hnt = pb.tile([P, CW], BF16, tag="hnt")
        nc.gpsimd.tensor_mul(hnt[:], o_t[:], tct[:])
        nc.sync.dma_start(hnq.ap()[bt][:, half], hnt[:])

    # ---- main schedule
    for bt in range(NBT):
        moms[bt] = mp.tile([P, 16], F32, tag="mom", name=f"mom{bt}")
        if bt == 1:
            mb_start(0)
        for c in range(NC):
            gemm_chunk(bt, c)
            if bt == 1:
                mb_chunk(0, c)
                if c == NC - 2:
                    mb_half(0, 0)
                if c == NC - 1:
                    mb_half(0, 1)
    mb_start(1)
    for c in range(NC):
        mb_chunk(1, c)
    mb_half(1, 0)
    mb_half(1, 1)


_nc = None


def _get_nc():
    global _nc
    if _nc is None:
        _nc = build()
    return _nc


def _pack_k(a):
    """[K, C] -> [128, K//256, 2, C] with k = kc*256 + 2p + i (DoubleRow)."""
    K, C = a.shape
    return np.ascontiguousarray(
        a.reshape(K // 256, 128, 2, C).transpose(1, 0, 2, 3))


def _q8(a):
    return np.clip(a, -240.0, 240.0).astype(NP_F8)


def _row_flat(v):
    """[G, H] -> [RS] in chunk-major order (c = g*2 + half)."""
    return np.ascontiguousarray(v.reshape(G * 2, CW).reshape(-1))


def make_in_maps(src_x, h, c, src_meta, zh_w, zh_b, zx_w, zx_b, zb_w,
                 dh_w, dx_w, db_w, db_b, w_h, w_x, ln_w, ln_b):
    f32 = np.float32
    perm = list(PERM)
    w_h = w_h[perm]
    w_x = w_x[perm]
    dh_w = dh_w[perm]
    dx_w = dx_w[perm]
    db_w = db_w[perm]
    db_b = db_b[perm]
    ln_w = ln_w[perm]
    ln_b = ln_b[perm]
    zh3 = zh_w.reshape(G, Z, Z)[perm]
    zx3 = zx_w.reshape(G, Z, Z)[perm]
    zb3 = zb_w.reshape(G, Z, Z)[perm]
    zh_b2 = zh_b.reshape(G, Z)[perm]
    zx_b2 = zx_b.reshape(G, Z)[perm]

    # hypernetwork fold: D_*(b) = src_meta[b] @ M_*[g] + bias row
    M_h = np.einsum("gzy,ghz->gyh", zh3, dh_w).astype(f32)
    M_x = np.einsum("gzy,ghz->gyh", zx3, dx_w).astype(f32)
    M_b = np.einsum("gzy,ghz->gyh", zb3, db_w).astype(f32)
    bdh = np.einsum("gz,ghz->gh", zh_b2, dh_w).astype(f32)
    bdx = np.einsum("gz,ghz->gh", zx_b2, dx_w).astype(f32)

    # replicated (per-core-identical) weight uploads
    def wpack(w):
        out = np.empty((NC, 128, KC, 2, CW), f32)
        for cidx in range(NC):
            g, half = cidx // 2, cidx % 2
            blk = w[g, half * CW:(half + 1) * CW, :]          # [CW, IN]
            out[cidx] = _pack_k(np.ascontiguousarray(blk.T))  # [IN, CW] packed
        return _q8(out * SW)

    def mpack(M, scale):
        out = np.empty((NC, 128, 2, CW), f32)
        for cidx in range(NC):
            g, half = cidx // 2, cidx % 2
            out[cidx] = _pack_k(M[g][:, half * CW:(half + 1) * CW])[:, 0]
        return _q8(out * scale)

    whq = wpack(w_h)
    wxq = wpack(w_x)
    # combined meta-GEMM weights [NC, 128, 3, 2, CW]: j = (h, x, b)
    m3q = np.stack([mpack(M_h, SM), mpack(M_x, SM), mpack(M_b, SM * SW)],
                   axis=2)
    # D_H/D_X rows ride at scale SM (they multiply W at scale SW -> SM*SW);
    # the additive D_B path carries the full SM*SW scale itself.
    rowq = np.concatenate([_row_flat(bdh * SM), _row_flat(bdx * SM),
                           _row_flat((db_b * SM * SW).astype(f32))])[None, :] \
        .astype(NP_BF)
    lnwq = np.ascontiguousarray(
        np.broadcast_to(_row_flat(ln_w)[None, :], (128, RS))).astype(NP_BF)
    # RZ: K=5 affine table. AFF = LT.T @ RZ with LT rows = [nmrs.T; ones]:
    # row j<4 holds lnw masked to gate j's chunks, row 4 holds lnb.
    rzq = np.zeros((5, RS), f32)
    lnw_f = _row_flat(ln_w)
    lnb_f = _row_flat(ln_b)
    for cidx in range(NC):
        g = cidx // 2
        sl = slice(cidx * CW, (cidx + 1) * CW)
        rzq[g, sl] = lnw_f[sl]
        rzq[4, sl] = lnb_f[sl]
    rzq = rzq.astype(NP_BF)
    idq = np.eye(128, dtype=NP_BF)

    xT = np.ascontiguousarray(src_x.T.astype(f32, copy=False))
    hT = np.ascontiguousarray(h.T.astype(f32, copy=False))
    mT = np.ascontiguousarray(src_meta.T.astype(f32, copy=False))

    in_maps = []
    for ci in range(NCORES):
        r0 = ci * BSH

        def actpack(aT):  # [K, B] slice -> [NBT, 128, K//256, 2, BT] fp8
            out = np.empty((NBT, 128, aT.shape[0] // 256, 2, BT), f32)
            for bt in range(NBT):
                out[bt] = _pack_k(aT[:, r0 + bt * BT:r0 + (bt + 1) * BT])
            return _q8(out)

        c_sl = c[r0:r0 + BSH].reshape(NBT, 128, 2, CW)
        in_maps.append({
            "xq": actpack(xT), "hq": actpack(hT),
            "mq": actpack(mT)[:, :, 0],
            "cq": c_sl.astype(NP_BF),
            "whq": whq, "wxq": wxq, "m3q": m3q,
            "rowq": rowq, "lnwq": lnwq, "rzq": rzq, "idq": idq,
        })
    return in_maps


def run(inputs, trace=False):
    nc = _get_nc()
    in_maps = make_in_maps(**inputs)
    res = run_bass_kernel_spmd(nc, in_maps, core_ids=list(range(NCORES)),
                               trace=trace)
    h_next = np.empty((B, H), np.float32)
    c_next = np.empty((B, H), np.float32)
    for ci in range(NCORES):
        rows = slice(ci * BSH, (ci + 1) * BSH)
        h_next[rows] = res.results[ci]["hnq"].reshape(BSH, H).astype(np.float32)
        c_next[rows] = res.results[ci]["cnq"].reshape(BSH, H).astype(np.float32)
    return (h_next, c_next), res


def kernel(**inputs):
    (h_next, c_next), _ = run(inputs, trace=False)
    return (h_next, c_next)
